# revision 1
# baseline (speedup 1.0000x reference)
"""2-layer GAT on 8 Trainium2 NeuronCores (Bass/Tile, SPMD).

Sharding: destination nodes i are partitioned across the 8 cores (512 rows
each); each core computes softmax + aggregation over all N=4096 sources for
its slice, both layers. The layer-1 projection g = x @ W1 is computed
replicated on every core (cheaper than AllGather + DRAM bounce; PE is
otherwise idle during the DMA-bound start). The only collective is a small
AllGather of g2_aug = elu(h) @ [W2 | W2.a2_dst] ([4096, 66] bf16) between the
layers. Layer-1 score terms s,t are O(N*heads) and precomputed on host as
x @ (W1 . a); s_i is softmax-invariant so bf16 everywhere is safe, only t_j
needs f32.

Per (j-tile, head) the score pipeline is:
  el = lrelu((s_rep + t_j) + nonadj*(-1e9))   (one fused custom DVE op,
                                               u8 mask, bf16 out)
  p  = exp(el)                                (ACT, head-pair-wide calls;
                                               masked entries underflow to 0)
  out[i,f], Z[i] += p^T @ [g | 1]             (PE: lhsT = p^T chunk, rhs has a
                                               ones column so Z rides along;
                                               f32 PSUM accum over 32 j-tiles)
then out/Z per head (ACT Copy with scale=1/Z), mean + ELU, layer 2 the same
shape but with g2 stationary (out^T in PSUM); the final divide-by-Z and
transpose happen on host from the returned [CLS+1, 512] raw slices.
"""

import numpy as np
import ml_dtypes

import concourse.bass as bass
import concourse.bacc as bacc
import concourse.mybir as mybir
import concourse.tile as tile
from concourse.bass_utils import run_bass_kernel_spmd
from concourse.masks import make_identity

N = 4096
IN = 256
HID = 256
HEADS = 4
CLS = 64
SLOPE = 0.2
NCORES = 8
IS = N // NCORES          # 512 destination rows per core
ICHUNKS = IS // 128       # 4
JT = N // 128             # 32 source-node tiles
NEG = -1.0e9

F32 = mybir.dt.float32
BF16 = mybir.dt.bfloat16
ADD = mybir.AluOpType.add
MULT = mybir.AluOpType.mult
MAX = mybir.AluOpType.max
AF = mybir.ActivationFunctionType

BF = ml_dtypes.bfloat16

# ---- custom fused DVE op: out = lrelu((in0 + s0) + in1) --------------------
import concourse.dve_ops as _dve_ops
from concourse.dve_spec import Spec as _Spec, Src0 as _Src0, Src1 as _Src1, \
    C0 as _C0, C2 as _C2, maxx as _maxx, lower as _dve_lower, _has_src1
from concourse.dve_uop import DveOpSpec as _DveOpSpec


def _gat_edge_ref(in0, in1, s0, s1, imm2):
    z = (in0.astype(np.float32) + s0) + in1
    return np.maximum(z, z * imm2).astype(np.float32)


def _gat_edge2_ref(in0, in1, s0, s1, imm2):
    z = (in0.astype(np.float32) + s0) + in1.astype(np.float32) * s1
    return np.maximum(z, z * imm2).astype(np.float32)


def _register(name, spec):
    if name in _dve_ops._SUB_OPCODE_FOR_NAME:
        return next(o for o in _dve_ops.OPS if o.name == name)
    opcode = _dve_ops._CUSTOM_DVE_ROW_BASE + len(_dve_ops.OPS)
    assert opcode < 0x20
    shas = {}
    for ver in ("v3", "v4"):
        s = _DveOpSpec(name=name, opcode=opcode,
                       uops=_dve_lower(spec, ver=ver), rd1_en=_has_src1(spec))
        shas[ver] = s.sha(ver)
    op = _dve_ops.DveOp(name, spec, subdim=False, uops_sha=shas)
    _dve_ops.OPS.append(op)
    _dve_ops._SUB_OPCODE_FOR_NAME[name] = opcode
    return op


_z1 = (_Src0 + _C0) + _Src1
GAT_EDGE = _register("GAT_EDGE",
                     _Spec(body=_maxx(_z1, _z1 * _C2), reference=_gat_edge_ref))
from concourse.dve_spec import C1 as _C1
_z2 = (_Src0 + _C0) + _Src1 * _C1
GAT_EDGE2 = _register("GAT_EDGE2",
                      _Spec(body=_maxx(_z2, _z2 * _C2), reference=_gat_edge2_ref))

_NC_CACHE = None


def build(reps=1, collectives=True):
    nc = bacc.Bacc("TRN2", target_bir_lowering=False, debug=False,
                   num_devices=NCORES if collectives else 1)

    xt = nc.dram_tensor("xt", [IN, N], BF16, kind="ExternalInput")
    w1 = nc.dram_tensor("w1", [IN, HEADS * HID], BF16, kind="ExternalInput")
    srep = nc.dram_tensor("srep", [128, HEADS * IS], BF16, kind="ExternalInput")
    t1 = nc.dram_tensor("t1", [N, HEADS], F32, kind="ExternalInput")
    maskt = nc.dram_tensor("maskt", [N, IS], mybir.dt.uint8, kind="ExternalInput")
    w2a = nc.dram_tensor("w2a", [HID, CLS + 2], BF16, kind="ExternalInput")
    y = nc.dram_tensor("y", [CLS + 1, IS], F32, kind="ExternalOutput")

    gath2 = [nc.dram_tensor(f"gath2_{r}", [N, CLS + 2], BF16,
                            kind="Internal", addr_space="Shared") for r in range(reps)]

    groups = [list(range(NCORES))]

    with tile.TileContext(nc) as tc:
        with (
            tc.tile_pool(name="sb", bufs=1) as sb,        # persistent tiles
            tc.tile_pool(name="wk", bufs=3) as wk,        # rotating work tiles
            tc.tile_pool(name="ps", bufs=8, space="PSUM") as ps,
            tc.tile_pool(name="dram", bufs=1, space="DRAM") as dram,
        ):
            # ---- resident inputs -------------------------------------------------
            ident = sb.tile([128, 128], BF16, tag="ident", name="ident")
            make_identity(nc, ident[:])

            xt_sb = [sb.tile([128, N], BF16, tag=f"xt{k}", name=f"xt{k}") for k in range(2)]
            for k in range(2):
                nc.sync.dma_start(xt_sb[k][:], xt[k * 128:(k + 1) * 128, :])
            w1_sb = [sb.tile([128, HEADS * HID], BF16, tag=f"w1{k}", name=f"w1s{k}") for k in range(2)]
            for k in range(2):
                nc.sync.dma_start(w1_sb[k][:], w1[k * 128:(k + 1) * 128, :])
            w2a_sb = [sb.tile([128, CLS + 2], BF16, tag=f"w2a{k}", name=f"w2as{k}") for k in range(2)]
            for k in range(2):
                nc.sync.dma_start(w2a_sb[k][:], w2a[k * 128:(k + 1) * 128, :])
            srep_sb = sb.tile([128, HEADS * IS], BF16, tag="srep", name="sreps")
            nc.sync.dma_start(srep_sb[:], srep[:, :])
            t1_sb = [sb.tile([128, HEADS], F32, tag=f"t1{j}", name=f"t1s{j}") for j in range(JT)]
            for j in range(JT):
                nc.sync.dma_start(t1_sb[j][:], t1[j * 128:(j + 1) * 128, :])
            mask_sb = [sb.tile([128, IS], mybir.dt.uint8, tag=f"mk{j}", name=f"mk{j}") for j in range(JT)]
            for j in range(JT):
                nc.sync.dma_start(mask_sb[j][:], maskt[j * 128:(j + 1) * 128, :])

            for rep in range(reps):
                # ---- layer-1 projection, replicated: every core computes full G.
                # G layout per j-tile: [g_h0(256) | 1 | g_h1(256) | 1 | ...] = 1028 cols
                g_sb = [sb.tile([128, HEADS * (HID + 1)], BF16, tag=f"g{j}", name=f"g{j}")
                        for j in range(JT)]
                jt_order = list(range(JT))
                for j in range(JT):
                    for nh in range(2):
                        pj = ps.tile([128, 512], F32, tag="ps", name="pj")
                        for k in range(2):
                            nc.tensor.matmul(
                                pj[:],
                                lhsT=xt_sb[k][:, j * 128:(j + 1) * 128],
                                rhs=w1_sb[k][:, nh * 512:(nh + 1) * 512],
                                start=(k == 0), stop=(k == 1),
                            )
                        # strided copy: PSUM [128, 2x256] -> g cols (2nh)*257, (2nh+1)*257
                        dst = g_sb[j][:, 2 * nh * 257:2 * nh * 257 + 514]
                        dst = dst.rearrange("p (b c) -> p b c", c=257)[:, :, 0:HID]
                        srcv = pj[:].rearrange("p (b c) -> p b c", c=HID)
                        if nh == 0:
                            nc.vector.tensor_copy(dst, srcv)
                        else:
                            nc.scalar.copy(dst, srcv)
                    for h in range(HEADS):
                        nc.vector.memset(
                            g_sb[j][:, h * 257 + HID:h * 257 + HID + 1], 1.0)

                # ---- layer-1 attention ----------------------------------------------
                contrib = {}
                for hp in range(2):                       # head pairs
                    heads = (2 * hp, 2 * hp + 1)
                    agg = {}
                    for h in heads:
                        for m in range(ICHUNKS):
                            agg[h, m] = ps.tile([128, HID + 1], F32, tag="ps", name=f"agg{h}_{m}")
                    for jn, j in enumerate(jt_order):
                        el = wk.tile([128, 2 * IS], BF16, tag="el", name="el", bufs=12)
                        for hi, h in enumerate(heads):
                            nc.vector._custom_dve(
                                GAT_EDGE2,
                                out=el[:, hi * IS:(hi + 1) * IS],
                                in0=srep_sb[:, h * IS:(h + 1) * IS],
                                in1=mask_sb[j][:],
                                s0=t1_sb[j][:, h:h + 1],
                                s1=NEG,
                                imm2=SLOPE,
                            )
                        p = wk.tile([128, 2 * IS], BF16, tag="p", name="p", bufs=12)
                        nc.scalar.activation(p[:], el[:], AF.Exp)
                        for hi, h in enumerate(heads):
                            for m in range(ICHUNKS):
                                nc.tensor.matmul(
                                    agg[h, m][:],
                                    lhsT=p[:, hi * IS + m * 128:hi * IS + (m + 1) * 128],
                                    rhs=g_sb[j][:, h * (HID + 1):(h + 1) * (HID + 1)],
                                    start=(jn == 0), stop=(jn == JT - 1),
                                )
                    # normalize: contrib = (agg / Z) * 0.25
                    for h in heads:
                        for m in range(ICHUNKS):
                            rz = wk.tile([128, 1], F32, tag="rz", name="rz")
                            nc.vector.reciprocal(rz[:], agg[h, m][:, HID:HID + 1])
                            ct = sb.tile([128, HID], F32, tag=f"ct{h}_{m}", name=f"ct{h}_{m}")
                            nc.scalar.activation(ct[:], agg[h, m][:, 0:HID],
                                                 AF.Copy, bias=0.0, scale=rz[:])
                            contrib[h, m] = ct

                # ---- head mean + ELU + g2_aug ---------------------------------------
                bounce2 = dram.tile([IS, CLS + 2], BF16, tag="b2", name="b2")
                ht_sb = [sb.tile([128, IS], BF16, tag=f"ht{k}", name=f"ht{k}") for k in range(2)]
                s2own = sb.tile([128, ICHUNKS], F32, tag="s2own", name="s2own")
                for m in range(ICHUNKS):
                    a0 = wk.tile([128, HID], F32, tag="ha", name="ha")
                    nc.vector.tensor_tensor(a0[:], contrib[0, m][:], contrib[1, m][:], ADD)
                    a1 = wk.tile([128, HID], F32, tag="hb", name="hb")
                    nc.vector.tensor_tensor(a1[:], contrib[2, m][:], contrib[3, m][:], ADD)
                    hm = wk.tile([128, HID], F32, tag="hm", name="hm")
                    nc.vector.tensor_tensor(hm[:], a0[:], a1[:], ADD)
                    # ELU on hm/4: r = relu(hm/4); u = exp(hm/4 - r); helu = (r-1)+u
                    r = wk.tile([128, HID], F32, tag="hr", name="hr")
                    nc.scalar.activation(r[:], hm[:], AF.Relu, bias=0.0, scale=0.25)
                    mn = wk.tile([128, HID], F32, tag="hn", name="hn")
                    nc.vector.scalar_tensor_tensor(
                        out=mn[:], in0=hm[:], scalar=0.25, in1=r[:],
                        op0=MULT, op1=mybir.AluOpType.subtract)
                    u = wk.tile([128, HID], F32, tag="hu", name="hu")
                    nc.scalar.activation(u[:], mn[:], AF.Exp)
                    helu = wk.tile([128, HID], BF16, tag="helu", name="helu")
                    nc.vector.scalar_tensor_tensor(
                        out=helu[:], in0=r[:], scalar=-1.0, in1=u[:], op0=ADD, op1=ADD)
                    # transpose helu into ht_sb (layer-2 lhsT)
                    for k in range(2):
                        pt = ps.tile([128, 128], BF16, tag="ps", name="pt")
                        nc.tensor.transpose(pt[:], helu[:, k * 128:(k + 1) * 128], ident[:])
                        nc.vector.tensor_copy(ht_sb[k][:, m * 128:(m + 1) * 128], pt[:])
                for m in range(ICHUNKS):
                    pg = ps.tile([128, CLS + 2], F32, tag="ps", name="pg")
                    for k in range(2):
                        nc.tensor.matmul(
                            pg[:], lhsT=ht_sb[k][:, m * 128:(m + 1) * 128],
                            rhs=w2a_sb[k][:], start=(k == 0), stop=(k == 1),
                        )
                    ag2 = wk.tile([128, CLS + 2], BF16, tag="ag2", name="ag2")
                    nc.vector.tensor_copy(ag2[:, 0:CLS], pg[:, 0:CLS])
                    nc.vector.memset(ag2[:, CLS:CLS + 1], 1.0)
                    nc.vector.tensor_copy(ag2[:, CLS + 1:CLS + 2], pg[:, CLS:CLS + 1])
                    nc.vector.tensor_copy(s2own[:, m:m + 1], pg[:, CLS + 1:CLS + 2])
                    nc.sync.dma_start(bounce2[m * 128:(m + 1) * 128, :], ag2[:])
                if collectives:
                    nc.gpsimd.collective_compute(
                        "AllGather", mybir.AluOpType.bypass, replica_groups=groups,
                        ins=[bounce2[:, :]], outs=[gath2[rep][:, :]],
                    )
                else:
                    nc.gpsimd.dma_start(gath2[rep][0:IS, :], bounce2[:, :])
                g2_sb = [sb.tile([128, CLS + 2], BF16, tag=f"g2_{j}", name=f"g2_{j}") for j in range(JT)]
                t2_sb = sb.tile([128, JT], F32, tag="t2", name="t2s")
                for j in range(JT):
                    nc.sync.dma_start(g2_sb[j][:], gath2[rep][j * 128:(j + 1) * 128, :])
                    nc.vector.tensor_copy(t2_sb[:, j:j + 1], g2_sb[j][:, CLS + 1:CLS + 2])

                # ---- s2 broadcast: [512] column -> [128, 512] row-replicated --------
                s2bf = wk.tile([128, ICHUNKS], BF16, tag="s2bf", name="s2bf")
                nc.vector.tensor_copy(s2bf[:], s2own[:])
                pt2 = ps.tile([1, IS], BF16, tag="ps", name="pt2")
                for m in range(ICHUNKS):
                    nc.tensor.transpose(
                        pt2[0:1, m * 128:(m + 1) * 128], s2bf[:, m:m + 1], ident[:])
                s2t = sb.tile([1, IS], BF16, tag="s2t", name="s2t")
                nc.vector.tensor_copy(s2t[:], pt2[:])
                ones1 = sb.tile([1, 128], BF16, tag="ones1", name="ones1")
                nc.vector.memset(ones1[:], 1.0)
                pr = ps.tile([128, IS], F32, tag="ps", name="pr")
                nc.tensor.matmul(pr[:], lhsT=ones1[:], rhs=s2t[:], start=True, stop=True)
                s2rep = sb.tile([128, IS], BF16, tag="s2rep", name="s2rep")
                nc.vector.tensor_copy(s2rep[:], pr[:])

                # ---- layer-2 attention ----------------------------------------------
                agg2t = ps.tile([CLS + 1, 512], F32, tag="ps", name="agg2t")
                for jp in range(JT // 2):
                    el2 = wk.tile([128, 2 * IS], BF16, tag="el", name="el2", bufs=12)
                    for d in range(2):
                        j = 2 * jp + d
                        nc.vector._custom_dve(
                            GAT_EDGE2, out=el2[:, d * IS:(d + 1) * IS], in0=s2rep[:],
                            in1=mask_sb[j][:], s0=t2_sb[:, j:j + 1], s1=NEG, imm2=SLOPE)
                    p2 = wk.tile([128, 2 * IS], BF16, tag="p", name="p2", bufs=12)
                    nc.scalar.activation(p2[:], el2[:], AF.Exp)
                    for d in range(2):
                        j = 2 * jp + d
                        nc.tensor.matmul(
                            agg2t[:], lhsT=g2_sb[j][:, 0:CLS + 1],
                            rhs=p2[:, d * IS:(d + 1) * IS],
                            start=(j == 0), stop=(j == JT - 1),
                        )
                yt_sb = wk.tile([CLS + 1, 512], F32, tag="yt", name="yt")
                nc.vector.tensor_copy(yt_sb[:], agg2t[:])
                nc.sync.dma_start(y[:, :], yt_sb[:])


    nc.compile()
    return nc


def _get_nc():
    global _NC_CACHE
    if _NC_CACHE is None:
        _NC_CACHE = build()
    return _NC_CACHE


def kernel(x, adj_mat, W1, a1_src, a1_dst, W2, a2_src, a2_dst):
    x = np.asarray(x, dtype=np.float32)
    adj = np.asarray(adj_mat, dtype=bool)
    W1 = np.asarray(W1, dtype=np.float32)
    a1_src = np.asarray(a1_src, dtype=np.float32)
    a1_dst = np.asarray(a1_dst, dtype=np.float32)
    W2 = np.asarray(W2, dtype=np.float32)
    a2_src = np.asarray(a2_src, dtype=np.float32)
    a2_dst = np.asarray(a2_dst, dtype=np.float32)

    # host-side tiny precomputation (O(N*IN) matmuls with 8-col outputs)
    W1r = W1.astype(np.float64).reshape(IN, HEADS, HID)
    w1s = np.einsum("khf,f->kh", W1r, a1_src.astype(np.float64))
    w1d = np.einsum("khf,f->kh", W1r, a1_dst.astype(np.float64))
    xd = x.astype(np.float64)
    s1 = (xd @ w1s).astype(np.float32)          # [N, HEADS]
    t1 = (xd @ w1d).astype(np.float32)          # [N, HEADS]
    w2aug = np.concatenate(
        [W2, (W2.astype(np.float64) @ a2_dst.astype(np.float64))[:, None].astype(np.float32),
         (W2.astype(np.float64) @ a2_src.astype(np.float64))[:, None].astype(np.float32)],
        axis=1,
    )                                            # [HID, CLS+2]: g2 | t2 | s2
    mask_neg = (~adj).T.astype(np.uint8)                          # [N(j), N(i)]
    xt_all = np.ascontiguousarray(x.T).astype(BF)                  # [IN, N]
    w1_bf = W1.astype(BF)
    w2a_bf = w2aug.astype(BF)

    in_maps = []
    for c in range(NCORES):
        isl = slice(c * IS, (c + 1) * IS)
        srep_c = np.broadcast_to(
            np.ascontiguousarray(s1[isl].T).reshape(1, HEADS * IS), (128, HEADS * IS)
        ).astype(BF)
        in_maps.append({
            "xt": xt_all,
            "w1": w1_bf,
            "srep": np.ascontiguousarray(srep_c),
            "t1": t1,
            "maskt": np.ascontiguousarray(mask_neg[:, isl]),
            "w2a": w2a_bf,
        })

    global _last_in_maps
    _last_in_maps = in_maps
    nc = _get_nc()
    res = run_bass_kernel_spmd(nc, in_maps, core_ids=list(range(NCORES)))
    outs = []
    for c in range(NCORES):
        raw = res.results[c]["y"]        # [CLS+1, IS]: rows 0:CLS unnorm, row CLS = Z
        outs.append((raw[0:CLS] / raw[CLS:CLS + 1]).T)
    return np.concatenate(outs, axis=0).astype(np.float32)

